# revision 1
# baseline (speedup 1.0000x reference)
"""Fused masked-attention kernel for Trainium2, data-parallel over batch on 8 cores.

v15 design notes (all per core; one batch element per core):
- Host layouts are pre-shuffled so every DMA descriptor is a >=8KB contiguous
  per-partition run (1KB packets capped the DMA fleet rate in the baseline).
- The bool mask ships as raw f16 {0,1} (1 = allowed) over plain HWDGE DMA and
  is applied with a uniform all-f16 tensor_mul on DVE, which runs in the DVE
  2x mode (~691ns/1024 elems vs 1274ns for u8-predicate copy_predicated).
  Uniformity matters: every mixed per-pair op scheme measured slower.
- Scores: row-tiled matmul pairs. QT/KT live duplicated in both partition
  halves (via [w|w]-duplicated projection weights), so chunk pairs run as two
  concurrent K=64 matmuls on disjoint PE row-groups (verified ~2x).
- V projects as V^T (stationary weights, one cheap LDWEIGHTS per e-chunk)
  and flips to [k, h] via PE transposes; q-block-0 projects right after the
  first K block so softmax work starts ~25us in; 110 warmup matmuls hold the
  PE HAM clock-gate open through the projection phase.
- AV accumulates f32 in PSUM over 32 k-chunks per 512-wide q block; V carries
  a ones column so PSUM row 64 is Z. Output ships unnormalized O^T+Z [65, L]
  f32; host does the divide + transpose.
- Steady state: ACT runs the pure exp stream (the wall, 1 elem/cycle/lane);
  DVE runs mask-mult + all evacuation copies; PE/Pool/DMA hold slack.
"""

import numpy as np

import concourse.bass as bass
import concourse.tile as tile
from concourse import bacc, mybir
from concourse import bass_utils

B, L, E, H = 8, 4096, 1024, 64
NCORES = 8
F32 = mybir.dt.float32
F16 = mybir.dt.float16
U8 = mybir.dt.uint8

LB = 512           # q-block and projection block width
NQB = L // LB      # 8
NCH = L // 128     # 32 k-chunks
NEC = E // 128     # 8 e-chunks


def build_nc():
    nc = bacc.Bacc(
        "TRN2",
        target_bir_lowering=False,
        debug=False,
        enable_asserts=False,
        num_devices=NCORES,
    )
    # host pre-shuffled layouts: per-partition runs are fully contiguous
    q2 = nc.dram_tensor("q2", [NQB, 128, NEC, LB], F16, kind="ExternalInput").ap()
    k2 = nc.dram_tensor("k2", [NQB, 128, NEC, LB], F16, kind="ExternalInput").ap()
    v2 = nc.dram_tensor("v2", [NQB, 128, NEC, LB], F16, kind="ExternalInput").ap()
    mu8 = nc.dram_tensor("mu8", [NQB, 128, NCH, LB], F16, kind="ExternalInput").ap()
    wqD = nc.dram_tensor("wqD", [E, 128], F16, kind="ExternalInput").ap()
    wkD = nc.dram_tensor("wkD", [E, 128], F16, kind="ExternalInput").ap()
    wvT = nc.dram_tensor("wvT", [E, H], F16, kind="ExternalInput").ap()
    ident = nc.dram_tensor("ident", [64, 64], F16, kind="ExternalInput").ap()
    out = nc.dram_tensor("out", [H + 1, L], F32, kind="ExternalOutput").ap()

    EXP = mybir.ActivationFunctionType.Exp

    with tile.TileContext(nc) as tc:
        with (
            tc.tile_pool(name="const", bufs=1) as constp,
            tc.tile_pool(name="persist", bufs=1) as persist,
            tc.tile_pool(name="kin", bufs=2) as kinp,
            tc.tile_pool(name="vin", bufs=2) as vinp,
            tc.tile_pool(name="qin", bufs=2) as qinp,
            tc.tile_pool(name="mpk", bufs=2) as mpool,
            tc.tile_pool(name="pt", bufs=10) as ptpool,
            tc.tile_pool(name="osb", bufs=2) as opool,
            tc.tile_pool(name="ps_st", bufs=2, space="PSUM") as ps_st,
            tc.tile_pool(name="ps_o", bufs=1, space="PSUM") as ps_o,
            tc.tile_pool(name="ps_pj", bufs=2, space="PSUM") as ps_pj,
        ):
            # ---- constants / weights ----
            wq_sb = constp.tile([128, NEC, 128], F16)
            wk_sb = constp.tile([128, NEC, 128], F16)
            wv_sb = constp.tile([128, NEC, H], F16)
            nc.sync.dma_start(wq_sb[:], wqD.rearrange("(c p) h -> p c h", p=128))
            nc.sync.dma_start(wk_sb[:], wkD.rearrange("(c p) h -> p c h", p=128))
            nc.sync.dma_start(wv_sb[:], wvT.rearrange("(c p) h -> p c h", p=128))
            id_sb = constp.tile([64, 64], F16)
            nc.sync.dma_start(id_sb[:], ident)

            # persistent projected tensors
            QT_sb = persist.tile([128, L], F16)   # rows 0:64 = Q^T, 64:128 = copy
            KT_sb = persist.tile([128, L], F16)
            VT_sb = persist.tile([64, L], F16)    # V^T staging
            V_sb = persist.tile([128, NCH, 128], F16)  # [k, h] + ones col 64, pad 0
            nc.vector.memset(V_sb[:, :, H : 128], 0.0)
            nc.vector.memset(V_sb[:, :, H : H + 1], 1.0)
            zero_sb = constp.tile([128, 1], F16)
            nc.vector.memset(zero_sb[:], 0.0)

            # mask for qb0
            mpk_sb0 = mpool.tile([128, NCH, LB], F16, tag="mpk")
            nc.sync.dma_start(mpk_sb0[:], mu8[0])

            # ---- PE warmup on weights (HAM) ----
            p_w = ps_st.tile([128, 1024], F32, tag="p_st")
            for w in range(110):
                nc.tensor.matmul(
                    p_w[:, 0:128], wq_sb[:, 0, :], wq_sb[:, 0, 0:128],
                    start=True, stop=True,
                )

            def proj_k_block(b, k_in):
                ls = b * LB
                p_pj = ps_pj.tile([128, LB], F32, tag="pj")
                for ec in range(NEC):
                    nc.tensor.matmul(
                        p_pj[:], wk_sb[:, ec, :], k_in[:, ec, :],
                        start=(ec == 0), stop=(ec == NEC - 1),
                    )
                nc.vector.tensor_copy(KT_sb[:, ls : ls + LB], p_pj[:])

            def proj_q_block(b, q_in):
                ls = b * LB
                p_pj = ps_pj.tile([128, LB], F32, tag="pj")
                for ec in range(NEC):
                    nc.tensor.matmul(
                        p_pj[:], wq_sb[:, ec, :], q_in[:, ec, :],
                        start=(ec == 0), stop=(ec == NEC - 1),
                    )
                nc.vector.tensor_copy(QT_sb[:, ls : ls + LB], p_pj[:])

            def proj_v_block(b, v_in):
                # project V^T (stationary weights: one cheap LDW per e-chunk)
                ls = b * LB
                p_pj = ps_pj.tile([128, LB], F32, tag="pj")
                for ec in range(NEC):
                    nc.tensor.matmul(
                        p_pj[0:H, :], wv_sb[:, ec, :], v_in[:, ec, :],
                        start=(ec == 0), stop=(ec == NEC - 1),
                    )
                nc.vector.tensor_copy(VT_sb[:, ls : ls + LB], p_pj[0:H, :])
                # flip to [k, h] via PE transposes
                for sub in range(4):
                    c = b * 4 + sub
                    p_tr = ps_o.tile([128, H], F16, tag="pjt")
                    nc.tensor.transpose(
                        p_tr[:], VT_sb[:, c * 128 : (c + 1) * 128], id_sb[:]
                    )
                    nc.vector.tensor_copy(V_sb[:, c, 0:H], p_tr[:])

            # ---- startup: stream K/V/q0 and project ----
            def load_one(pool, tag, src):
                t = pool.tile([128, NEC, LB], F16, tag=tag)
                nc.sync.dma_start(t[:], src)
                return t

            q0 = load_one(qinp, "qin", q2[0])
            for b in range(NQB):
                k_in = load_one(kinp, "kin", k2[b])
                v_in = load_one(vinp, "vin", v2[b])
                proj_k_block(b, k_in)
                if b == 0:
                    proj_q_block(0, q0)
                proj_v_block(b, v_in)

            # ---- main loop ----
            mtile = mpk_sb0
            q_next = None
            for qb in range(NQB):
                qs = qb * LB
                p_o = ps_o.tile([128, LB], F32, tag="p_o")
                m_next = None
                for g in range(16):
                    cA, cB = 2 * g, 2 * g + 1
                    ps = ps_st.tile([128, 1024], F32, tag="p_st")
                    # two concurrent K=64 row-tiled score matmuls (N=512)
                    nc.tensor.matmul(
                        ps[:, 0:512],
                        KT_sb[0:64, cA * 128 : (cA + 1) * 128],
                        QT_sb[0:64, qs : qs + LB],
                        start=True, stop=True,
                    )
                    nc.tensor.matmul(
                        ps[:, 512:1024],
                        KT_sb[64:128, cB * 128 : (cB + 1) * 128],
                        QT_sb[64:128, qs : qs + LB],
                        start=True, stop=True,
                    )
                    # exp on ACT
                    pt = ptpool.tile([128, 1024], F16, tag="pt")
                    nc.scalar.activation(pt[:], ps[:], EXP, scale=0.125)
                    # mask-mult, all-f16 SBUF operands (DVE 2x mode)
                    nc.vector.tensor_mul(
                        pt[:],
                        pt[:],
                        mtile[:, cA : cA + 2, :].rearrange("p c q -> p (c q)"),
                    )
                    # prefetch hooks
                    if g == 1 and qb + 1 < NQB:
                        m_next = mpool.tile([128, NCH, LB], F16, tag="mpk")
                        nc.sync.dma_start(m_next[:], mu8[qb + 1])
                    if g == 4 and qb + 1 < NQB:
                        q_next = load_one(qinp, "qin", q2[qb + 1])
                    if g == 8 and qb + 1 < NQB:
                        proj_q_block(qb + 1, q_next)
                    # AV: accumulate both chunks
                    nc.tensor.matmul(
                        p_o[:], V_sb[:, cA, :], pt[:, 0:512],
                        start=(g == 0), stop=False,
                    )
                    nc.tensor.matmul(
                        p_o[:], V_sb[:, cB, :], pt[:, 512:1024],
                        start=False, stop=(g == 15),
                    )
                # epilogue: ship unnormalized O^T + Z row
                o_sb = opool.tile([H + 1, LB], F32, tag="osb")
                nc.vector.tensor_copy(o_sb[:], p_o[0 : H + 1, :])
                nc.sync.dma_start(out[:, qs : qs + LB], o_sb[:])
                mtile = m_next
    nc.compile()
    return nc


_NC_CACHE = {}


def _shuffle_pcl(xT):
    """xT: [E, L] -> [NQB, 128, NEC, LB], so partition p of block b holds
    e-rows {c*128+p} as contiguous 512-col runs."""
    a = xT.reshape(NEC, 128, NQB, LB)      # [c, p, b, l']
    return np.ascontiguousarray(a.transpose(2, 1, 0, 3))


def _shuffle_mask(forb_b):
    """forb_b: [L, L] bool (True = forbidden), indexed [q, k].
    Returns [NQB, 128, NCH, LB] u8: [qb, p, c, q'] = forb[qb*512+q', c*128+p]."""
    A = forb_b.T.reshape(NCH, 128, NQB, LB)  # [c, p, qb, q']
    return np.ascontiguousarray(A.transpose(2, 1, 0, 3)).astype(np.uint8)


def kernel(query, key, value, mask, WQ, WK, WV):
    if "nc" not in _NC_CACHE:
        _NC_CACHE["nc"] = build_nc()
    nc = _NC_CACHE["nc"]

    wqT = np.asarray(WQ, dtype=np.float16).T  # [E, H]
    wkT = np.asarray(WK, dtype=np.float16).T
    wvT = np.ascontiguousarray(np.asarray(WV, dtype=np.float16).T)
    wqD = np.ascontiguousarray(np.concatenate([wqT, wqT], axis=1))
    wkD = np.ascontiguousarray(np.concatenate([wkT, wkT], axis=1))
    idn = np.eye(64, dtype=np.float16)
    forb = np.asarray(mask)  # [B, L, L], True where forbidden
    in_maps = []
    for b in range(B):
        in_maps.append(
            {
                "q2": _shuffle_pcl(np.asarray(query[b], dtype=np.float16).T),
                "k2": _shuffle_pcl(np.asarray(key[b], dtype=np.float16).T),
                "v2": _shuffle_pcl(np.asarray(value[b], dtype=np.float16).T),
                "mu8": (1 - _shuffle_mask(forb[b])).astype(np.float16),
                "wqD": wqD,
                "wkD": wkD,
                "wvT": wvT,
                "ident": idn,
            }
        )
    res = bass_utils.run_bass_kernel_spmd(nc, in_maps, core_ids=list(range(NCORES)))
    outs = []
    for b in range(B):
        ot = res.results[b]["out"].astype(np.float64)  # [65, L]
        o = (ot[0:H] / ot[H : H + 1]).T  # [L, H]
        outs.append(o.astype(np.float32))
    return np.stack(outs, axis=0)


if __name__ == "__main__":
    rng = np.random.default_rng(0)
    q = rng.standard_normal((B, L, E), dtype=np.float32)
    k = rng.standard_normal((B, L, E), dtype=np.float32)
    v = rng.standard_normal((B, L, E), dtype=np.float32)
    m = rng.integers(0, 2, size=(B, L, L)).astype(bool)
    s = 1.0 / np.sqrt(E)
    wq = rng.uniform(-s, s, size=(H, E)).astype(np.float32)
    wk = rng.uniform(-s, s, size=(H, E)).astype(np.float32)
    wv = rng.uniform(-s, s, size=(H, E)).astype(np.float32)
    o = kernel(query=q, key=k, value=v, mask=m, WQ=wq, WK=wk, WV=wv)
    print(o.shape, o.dtype)

